# revision 23
# baseline (speedup 1.0000x reference)
"""Deformable Transformer encoder layer on 8 Trainium2 NeuronCores (Bass/Tile).

Sharding: core k handles batch b=k//2, query half k%2 (2720 queries each);
the full layer runs per-core with no collectives, host stacks the slices.

Per-core dataflow (channel-on-partition "transposed" layout; channel-PAIR
packing: partition p=16h+j holds channels (32h+2j, 32h+2j+1) as f16 pairs
so one head = 16 partitions = one GPSIMD Q7 core):
  PE transposes src/pos/ref -> value proj via column-permuted w_val (fp16
  matmul, strided drain) -> v16P [128, LEN, 2] -> VPQ quad table: entry
  e=(y0+1)*W+x holds d=4 f32 = 4 bilinear corners x channel-pair, with
  zero-pad rows at y=-1 / y=H -> sampling offsets/attn logits via PE
  matmuls with biases folded in -> single clamped y0-index + 4 corner
  weights per point on DVE/ACT in [(l,p,h), q] tiles (floor via int16
  round trip with +1024 shift) -> idx wrap-transpose (one stream per
  head; 3-dim partition-split DMAs) -> ONE GPSIMD ap_gather d=4 per
  (l,p,chunk) fetches all 4 corners for all 8 heads -> combine: PE
  broadcasts 4 corner weights to 16 partitions/head (sel-matmul), ACT
  drains duplicated to f16, DVE multiplies, PE identity-matmuls
  accumulate per channel-pair lane into even/odd PSUM -> out-proj via
  row-permuted w_out with stride-2 moving operands + LN + FFN + LN ->
  PE transpose back to row-major.

Self-contained: hardcodes all shapes; reads nothing from the problem dir.
"""
import sys
sys.path.insert(0, '/opt/trn_rl_repo')
import numpy as np
import ml_dtypes

import concourse.bass as bass
import concourse.mybir as mybir
import concourse.tile as tile
from concourse import bacc, library_config

f32 = mybir.dt.float32
f16 = mybir.dt.float16
i16 = mybir.dt.int16
AL = mybir.AluOpType
AF = mybir.ActivationFunctionType
AX = mybir.AxisListType

SPATIAL = [(64, 64), (32, 32), (16, 16), (8, 8)]
HWs = [h * w for h, w in SPATIAL]
LOFF = [0, 4096, 5120, 5376, 5440]
# quad-table: per level (Hl+1)*Wl entries of d=4 f32
NEL = [(h + 1) * w for h, w in SPATIAL]          # 4160, 1056, 272, 72
QOFF = [0]
for _n in NEL:
    QOFF.append(QOFF[-1] + _n * 4)               # f32 offsets; total 22240
LEN, B, C, H, L, P, DH, DFF = 5440, 4, 256, 8, 4, 4, 32, 1024
NQ = 2720
EPS = 1e-5
NCHUNK = [512, 512, 512, 512, 512, 160]
COFF = [0, 512, 1024, 1536, 2048, 2560]
NQT = 22  # ceil(2720/128)
SH = 1024.0  # floor-trick shift
# Convert rounding differs between CoreSim (truncate toward zero) and HW
# (round-half-even). floor(px)+SH == trunc(px+SH) == rhe(px+SH-0.5), so the
# convert input needs +0.5 in sim mode only.
FLOOR_SIM = False


def _ceil(a, b):
    return (a + b - 1) // b


def build_nc(repeat=1):
    nc = bacc.Bacc(None, target_bir_lowering=False, debug=False)

    src_full_d = nc.dram_tensor("src_full", [LEN, C], f32, kind="ExternalInput")
    srcq_d = nc.dram_tensor("srcq", [NQ, C], f32, kind="ExternalInput")
    posq_d = nc.dram_tensor("posq", [NQ, C], f32, kind="ExternalInput")
    refq_d = nc.dram_tensor("refq", [NQ, 8], f32, kind="ExternalInput")
    w_val_d = nc.dram_tensor("w_val", [C, C], f32, kind="ExternalInput")
    bvalT_d = nc.dram_tensor("bvalT", [128, 2], f32, kind="ExternalInput")
    w_off_d = nc.dram_tensor("w_offp", [C, C], f32, kind="ExternalInput")
    refsel_d = nc.dram_tensor("refsel", [16, C], f32, kind="ExternalInput")
    w_attn_d = nc.dram_tensor("w_attnp", [C, 128], f32, kind="ExternalInput")
    b_attn_d = nc.dram_tensor("b_attnp", [1, 128], f32, kind="ExternalInput")
    w_out16_d = nc.dram_tensor("w_out16", [C, C], f32, kind="ExternalInput")
    boutT_d = nc.dram_tensor("boutT", [128, 2], f32, kind="ExternalInput")
    g1_d = nc.dram_tensor("g1T", [128, 2], f32, kind="ExternalInput")
    be1_d = nc.dram_tensor("be1T", [128, 2], f32, kind="ExternalInput")
    g2_d = nc.dram_tensor("g2T", [128, 2], f32, kind="ExternalInput")
    be2_d = nc.dram_tensor("be2T", [128, 2], f32, kind="ExternalInput")
    w1_d = nc.dram_tensor("w1", [C, DFF], f32, kind="ExternalInput")
    b1T_d = nc.dram_tensor("b1T", [128, 8], f32, kind="ExternalInput")
    w2_d = nc.dram_tensor("w2", [DFF, C], f32, kind="ExternalInput")
    b2T_d = nc.dram_tensor("b2T", [128, 2], f32, kind="ExternalInput")
    ident_d = nc.dram_tensor("ident", [128, 128], f32, kind="ExternalInput")
    ident16_d = nc.dram_tensor("ident16", [128, 128], f32, kind="ExternalInput")
    bsel16_d = nc.dram_tensor("bsel16", [128, 16, 128], f32, kind="ExternalInput")
    # per-(l,p,h)-partition consts:
    # 0: Wl, 1: SH+Wl-1, 2: SH+Wl-2, 3: SH+Hl-1, 4: (SH-1)*Wl+SH
    pc_d = nc.dram_tensor("pconst", [128, 5], f32, kind="ExternalInput")
    out_d = nc.dram_tensor("out", [NQ, C], f32, kind="ExternalOutput")

    from contextlib import ExitStack
    with tile.TileContext(nc) as tc:
      for _rep in range(repeat):
       with ExitStack() as ctx:
        pool = lambda n, b: ctx.enter_context(tc.tile_pool(name=n, bufs=b))
        psum = lambda n, b: ctx.enter_context(
            tc.tile_pool(name=n, bufs=b, space="PSUM"))
        consts = pool("consts", 1)
        rowp = pool("rowp", 2)
        tp_ps = psum("tp_ps", 1)
        mm_ps = psum("mm_ps", 2)
        P_acc = tc.tile_pool(name="P_acc", bufs=1)
        p_acc = ctx.enter_context(P_acc)
        P_vpq = tc.tile_pool(name="P_vpq", bufs=1)
        p_vpq = P_vpq.__enter__()
        P_w4 = tc.tile_pool(name="P_w4", bufs=1)
        p_w4 = P_w4.__enter__()

        def cst(dram, shape, dtype=f32):
            t = consts.tile(shape, dtype, tag=dram.name + "_s", name=dram.name + "_s")
            nc.sync.dma_start(t[:], dram[:])
            return t

        def cstk(dram, nk, ncols, dtype=f32, pl=None):
            pl = pl or consts
            ts = []
            for kb in range(nk):
                t = pl.tile([128, ncols], dtype,
                            tag=f"{dram.name}_k{kb}", name=f"{dram.name}_k{kb}")
                if dtype == f16:
                    for jc in range(_ceil(ncols, 512)):
                        a, bwid = jc * 512, min(512, ncols - jc * 512)
                        tmp = rowp.tile([128, 512], f32, tag="cvtw",
                                        name="cvtw")
                        nc.sync.dma_start(
                            tmp[:, :bwid],
                            dram[128 * kb:128 * kb + 128, a:a + bwid])
                        nc.vector.tensor_copy(t[:, a:a + bwid], tmp[:, :bwid])
                else:
                    nc.sync.dma_start(t[:], dram[128 * kb:128 * kb + 128])
                ts.append(t)
            return ts

        ident = cst(ident_d, [128, 128])
        ident16 = consts.tile([128, 128], f16, tag="ident16", name="ident16")
        nc.vector.tensor_copy(ident16[:], ident[:])
        # bselP [128, 8, 128]: 64-block b=l//2, col si=(l%2)*4+p: row
        # (l%2)*32+p*8+h -> partitions 16h..16h+15 (base partition 0/64 only)
        bselP = consts.tile([128, 8, 128], f16, tag="bselP", name="bselP")
        for jc in range(2):
            bstmp = rowp.tile([128, 512], f32, tag="cvtw", name="bstmp")
            bdv = bsel16_d[:, 4 * jc:4 * jc + 4, :].rearrange(
                "p a b -> p (a b)")
            nc.sync.dma_start(bstmp[:], bdv)
            nc.vector.tensor_copy(
                bselP[:, 4 * jc:4 * jc + 4, :].rearrange("p a b -> p (a b)"),
                bstmp[:])
        pc = cst(pc_d, [128, 5])
        w_val = cstk(w_val_d, 2, C, f16)
        bvalT = cst(bvalT_d, [128, 2])
        w_offp = cstk(w_off_d, 2, C, f16)
        refsel = cst(refsel_d, [16, C])
        w_attnp = cstk(w_attn_d, 2, 128, f16)
        b_attnp = cst(b_attn_d, [1, 128])
        w_out16 = cstk(w_out16_d, 2, C, f16)
        boutT = cst(boutT_d, [128, 2])
        g1T = cst(g1_d, [128, 2])
        be1T = cst(be1_d, [128, 2])
        g2T = cst(g2_d, [128, 2])
        be2T = cst(be2_d, [128, 2])
        b1T = cst(b1T_d, [128, 8])
        b2T = cst(b2T_d, [128, 2])

        ones_col = consts.tile([128, 1], f32, tag="ones_col")
        nc.vector.memset(ones_col[:], 1.0)
        ones1x128 = consts.tile([1, 128], f32, tag="ones1x128")
        nc.vector.memset(ones1x128[:], 1.0)

        def mkconst(val, tag):
            t = consts.tile([128, 1], f32, tag=tag, name=tag)
            nc.vector.memset(t[:], val)
            return t

        c_eps1 = consts.tile([1, 1], f32, tag="c_eps1", name="c_eps1")
        nc.vector.memset(c_eps1[:], EPS)
        c_lo = mkconst(SH, "c_lo")          # shifted 0  (x0 >= 0 bound)
        c_lom1 = mkconst(SH - 1.0, "c_lom1")  # shifted -1 (x0 >= -1 bound)

        def bc(t, cn):
            return t[:, 0:1].to_broadcast([128, cn])

        def pcb(k, cn):
            return pc[:, k:k + 1].to_broadcast([128, cn])

        # ---------------- transposes ----------------
        def transpose_rows(dst_tiles, dram, nrows, add_dram=None):
            for i in range(_ceil(nrows, 128)):
                r0 = i * 128
                rn = min(128, nrows - r0)
                rt = rowp.tile([128, C], f32, tag="rows")
                nc.sync.dma_start(rt[:rn], dram[r0:r0 + rn])
                if add_dram is not None:
                    rt2 = rowp.tile([128, C], f32, tag="rows2")
                    nc.sync.dma_start(rt2[:rn], add_dram[r0:r0 + rn])
                    nc.vector.tensor_tensor(rt[:rn], rt[:rn], rt2[:rn],
                                            op=AL.add)
                for cb in range(2):
                    ps = tp_ps.tile([128, 128], f32, tag="tp")
                    nc.tensor.transpose(ps[:, :rn],
                                        rt[:rn, 128 * cb:128 * cb + 128],
                                        ident[:rn, :rn])
                    nc.scalar.copy(dst_tiles[cb][:, r0:r0 + rn], ps[:, :rn])

        VPQ = p_vpq.tile([128, QOFF[-1]], f32, tag="VPQ", name="VPQ")

        # ---------------- qT / refT9 transposes ----------------
        P_q = tc.tile_pool(name="P_q", bufs=1)
        p_q = P_q.__enter__()
        P_aw = tc.tile_pool(name="P_aw", bufs=1)
        p_aw = P_aw.__enter__()
        P_ref = tc.tile_pool(name="P_ref", bufs=1)
        p_ref = P_ref.__enter__()
        qT = [p_q.tile([128, NQ], f16, tag=f"qT{i}", name=f"qT{i}")
              for i in range(2)]
        transpose_rows(qT, srcq_d, NQ, add_dram=posq_d)
        refT9 = p_ref.tile([16, NQ], f32, tag="refT9", name="refT9")
        nc.vector.memset(refT9[:], 1.0)
        for i in range(NQT):
            r0 = i * 128
            rn = min(128, NQ - r0)
            rt = rowp.tile([128, 8], f32, tag="refrows")
            nc.sync.dma_start(rt[:rn], refq_d[r0:r0 + rn])
            ps = tp_ps.tile([128, 128], f32, tag="tp")
            nc.tensor.transpose(ps[:8, :rn], rt[:rn, :8], ident[:rn, :rn])
            nc.scalar.copy(refT9[0:8, r0:r0 + rn], ps[:8, :rn])

        # ---------------- attention softmax -> awT ----------------
        awT = p_aw.tile([128, NQ], f16, tag="awT", name="awT")
        SMP = tc.tile_pool(name="smp", bufs=3)
        smp = SMP.__enter__()
        for i in range(NQT):
            r0 = i * 128
            rn = min(128, NQ - r0)
            ps = mm_ps.tile([128, 128], f32, tag="mm")
            for kb in range(2):
                nc.tensor.matmul(ps[:rn], qT[kb][:, r0:r0 + rn],
                                 w_attnp[kb][:],
                                 start=(kb == 0), stop=False)
            nc.tensor.matmul(ps[:rn], ones1x128[:, :rn], b_attnp[:],
                             start=False, stop=True)
            aw = smp.tile([128, 128], f32, tag="aw")
            mx = smp.tile([128, 8], f32, tag="mx")
            sv = ps[:rn].rearrange("q (lp h) -> q h lp", h=8)
            av = aw[:rn].rearrange("q (lp h) -> q h lp", h=8)
            nc.vector.tensor_reduce(mx[:rn], sv, AX.X, op=AL.max)
            nc.vector.tensor_tensor(
                av, sv, mx[:rn].unsqueeze(2).to_broadcast([rn, 8, 16]),
                op=AL.subtract)
            nc.scalar.activation(aw[:rn], aw[:rn], AF.Exp)
            sm = smp.tile([128, 8], f32, tag="sm")
            nc.vector.tensor_reduce(sm[:rn], av, AX.X, op=AL.add)
            rc = smp.tile([128, 8], f32, tag="rc")
            nc.vector.reciprocal(rc[:rn], sm[:rn])
            nc.vector.tensor_tensor(
                av, av, rc[:rn].unsqueeze(2).to_broadcast([rn, 8, 16]),
                op=AL.mult)
            ps2 = tp_ps.tile([128, 128], f32, tag="tp")
            nc.tensor.transpose(ps2[:, :rn], aw[:rn], ident[:rn, :rn])
            nc.scalar.copy(awT[:, r0:r0 + rn], ps2[:, :rn])

        # ---------------- index/weight pipeline ----------------
        # W4 layout [128, q, row(y), slot(x)] f16 ; e16 [128, q] i16
        W4 = p_w4.tile([128, NQ, 2, 2], f16, tag="W4", name="W4")
        e16 = p_w4.tile([128, NQ], i16, tag="e16", name="e16")
        PIP = tc.tile_pool(name="pip", bufs=1)
        pip = PIP.__enter__()
        OFF_PS = tc.tile_pool(name="off_ps", bufs=1, space="PSUM")
        off_ps = OFF_PS.__enter__()

        for ci, cn in enumerate(NCHUNK):
            c0 = COFF[ci]
            pxy = []
            for comp in range(2):
                ps = off_ps.tile([128, 512], f32, tag=f"off{comp}")
                for kb in range(2):
                    nc.tensor.matmul(
                        ps[:, :cn],
                        w_offp[kb][:, 128 * comp:128 * comp + 128],
                        qT[kb][:, c0:c0 + cn], start=(kb == 0), stop=False)
                nc.tensor.matmul(ps[:, :cn],
                                 refsel[:, 128 * comp:128 * comp + 128],
                                 refT9[:, c0:c0 + cn], start=False, stop=True)
                pxy.append(ps)
            p5x, p5y = pxy  # = coord - 0.5 + SH

            def T(tag, dtype=f32):
                return pip.tile([128, 512], dtype, tag=tag, name=tag)

            def axis_weights(p5, kb_hi1):
                """z0f = floor(coord)+SH and frac weight wz1"""
                c16 = T("c16_" + kb_hi1, i16)
                if FLOOR_SIM:
                    ci_t = T("cvt_in")
                    nc.scalar.activation(ci_t[:, :cn], p5[:, :cn], AF.Copy,
                                         bias=0.5)
                    nc.vector.tensor_copy(c16[:, :cn], ci_t[:, :cn])
                else:
                    nc.vector.tensor_copy(c16[:, :cn], p5[:, :cn])
                z0f = T("z0f" + kb_hi1)
                nc.scalar.copy(z0f[:, :cn], c16[:, :cn])  # x0 + SH
                wz1 = T("wz1" + kb_hi1)
                nc.vector.scalar_tensor_tensor(wz1[:, :cn], p5[:, :cn], 0.5,
                                               z0f[:, :cn], op0=AL.add,
                                               op1=AL.subtract)
                return z0f, wz1

            # --- x ---
            x0f, wx1 = axis_weights(p5x, "x")
            wx0 = T("wx0")
            nc.scalar.activation(wx0[:, :cn], wx1[:, :cn], AF.Copy,
                                 bias=1.0, scale=-1.0)
            basex = T("basex")
            nc.vector.scalar_tensor_tensor(basex[:, :cn], x0f[:, :cn], SH,
                                           pcb(2, cn), op0=AL.max, op1=AL.min)
            dd = T("dd")
            nc.vector.tensor_tensor(dd[:, :cn], basex[:, :cn], x0f[:, :cn],
                                    op=AL.subtract)
            mA = T("mA")
            nc.scalar.activation(mA[:, :cn], dd[:, :cn], AF.Abs)
            nc.scalar.activation(mA[:, :cn], mA[:, :cn], AF.Copy,
                                 bias=1.0, scale=-1.0)
            t1 = T("t1")
            t2 = T("t2")
            wA_v = T("wA_v")
            nc.vector.tensor_tensor(t1[:, :cn], x0f[:, :cn], bc(c_lo, cn),
                                    op=AL.is_ge)
            nc.vector.tensor_tensor(wA_v[:, :cn], wx0[:, :cn], t1[:, :cn],
                                    op=AL.mult)
            nc.vector.tensor_tensor(t1[:, :cn], x0f[:, :cn], pcb(1, cn),
                                    op=AL.is_le)
            nc.vector.tensor_tensor(wA_v[:, :cn], wA_v[:, :cn], t1[:, :cn],
                                    op=AL.mult)
            wB_v = T("wB_v")
            nc.vector.tensor_tensor(t2[:, :cn], x0f[:, :cn], bc(c_lom1, cn),
                                    op=AL.is_ge)
            nc.vector.tensor_tensor(wB_v[:, :cn], wx1[:, :cn], t2[:, :cn],
                                    op=AL.mult)
            nc.vector.tensor_tensor(t2[:, :cn], x0f[:, :cn], pcb(2, cn),
                                    op=AL.is_le)
            nc.vector.tensor_tensor(wB_v[:, :cn], wB_v[:, :cn], t2[:, :cn],
                                    op=AL.mult)
            # wsA = mA*wA + relu(dd)*wB ; wsB = mA*wB - min(dd,0)*wA
            wsA = T("wsA")
            nc.vector.tensor_tensor(wsA[:, :cn], mA[:, :cn], wA_v[:, :cn],
                                    op=AL.mult)
            nc.vector.scalar_tensor_tensor(t1[:, :cn], dd[:, :cn], 0.0,
                                           wB_v[:, :cn], op0=AL.max,
                                           op1=AL.mult)
            nc.vector.tensor_tensor(wsA[:, :cn], wsA[:, :cn], t1[:, :cn],
                                    op=AL.add)
            wsB = T("wsB")
            nc.vector.tensor_tensor(wsB[:, :cn], mA[:, :cn], wB_v[:, :cn],
                                    op=AL.mult)
            nc.vector.scalar_tensor_tensor(t2[:, :cn], dd[:, :cn], 0.0,
                                           wA_v[:, :cn], op0=AL.min,
                                           op1=AL.mult)
            nc.vector.tensor_tensor(wsB[:, :cn], wsB[:, :cn], t2[:, :cn],
                                    op=AL.subtract)

            # --- y ---
            y0f, wy1 = axis_weights(p5y, "x")
            wy0 = T("wx0")
            nc.scalar.activation(wy0[:, :cn], wy1[:, :cn], AF.Copy,
                                 bias=1.0, scale=-1.0)
            # single clamped row index: y0 in [-1, Hl-1] (shifted)
            yc = T("yr0")
            nc.vector.scalar_tensor_tensor(yc[:, :cn], y0f[:, :cn], SH - 1.0,
                                           pcb(3, cn), op0=AL.max, op1=AL.min)
            wy0a = T("wA_v")
            nc.vector.tensor_tensor(t1[:, :cn], y0f[:, :cn], bc(c_lo, cn),
                                    op=AL.is_ge)
            nc.vector.tensor_tensor(wy0a[:, :cn], wy0[:, :cn], t1[:, :cn],
                                    op=AL.mult)
            nc.vector.tensor_tensor(t1[:, :cn], y0f[:, :cn], pcb(3, cn),
                                    op=AL.is_le)
            nc.vector.tensor_tensor(wy0a[:, :cn], wy0a[:, :cn], t1[:, :cn],
                                    op=AL.mult)
            nc.vector.tensor_tensor(wy0a[:, :cn], wy0a[:, :cn],
                                    awT[:, c0:c0 + cn], op=AL.mult)
            wy1a = T("wB_v")
            nc.vector.tensor_tensor(t2[:, :cn], y0f[:, :cn], bc(c_lom1, cn),
                                    op=AL.is_ge)
            nc.vector.tensor_tensor(wy1a[:, :cn], wy1[:, :cn], t2[:, :cn],
                                    op=AL.mult)
            # y0+1 <= Hl-1  <=>  y0f <= SH+Hl-2
            nc.vector.scalar_tensor_tensor(t2[:, :cn], pcb(3, cn), 1.0,
                                           y0f[:, :cn], op0=AL.subtract,
                                           op1=AL.is_ge)
            nc.vector.tensor_tensor(wy1a[:, :cn], wy1a[:, :cn], t2[:, :cn],
                                    op=AL.mult)
            nc.vector.tensor_tensor(wy1a[:, :cn], wy1a[:, :cn],
                                    awT[:, c0:c0 + cn], op=AL.mult)

            for (row, wya) in ((0, wy0a), (1, wy1a)):
                for (slot, wsx) in ((0, wsA), (1, wsB)):
                    nc.vector.tensor_tensor(
                        W4[:, c0:c0 + cn, row, slot], wsx[:, :cn],
                        wya[:, :cn], op=AL.mult)

            # quad-table index: e = yc*Wl + basex - ((SH-1)*Wl + SH)
            e = T("dd")
            nc.vector.scalar_tensor_tensor(e[:, :cn], yc[:, :cn], 0.0,
                                           pcb(0, cn), op0=AL.max,
                                           op1=AL.mult)
            nc.vector.tensor_tensor(e[:, :cn], e[:, :cn], basex[:, :cn],
                                    op=AL.add)
            nc.vector.tensor_tensor(e[:, :cn], e[:, :cn], pcb(4, cn),
                                    op=AL.subtract)
            ccols, cw0 = cn // 16, c0 // 16
            sv = e[:, :cn].rearrange("p (c w) -> p c w", w=16)
            dv = e16[:].rearrange(
                "p (w c) -> p c w", c=NQ // 16)[:, cw0:cw0 + ccols, :]
            nc.vector.tensor_copy(dv, sv)

        OFF_PS.__exit__(None, None, None)
        PIP.__exit__(None, None, None)
        SMP.__exit__(None, None, None)
        P_ref.__exit__(None, None, None)
        P_aw.__exit__(None, None, None)
        P_q.__exit__(None, None, None)

        # ---------------- value proj (fp16, channel-pair packed) ----------
        # Issued after the pipeline so the PE/DVE queues run the index
        # pipeline first; VPQ copies go on ACT (never stalls DVE).
        # v16P[p, e, d] = value[32*(p//16) + 2*(p%16) + d, e]
        P_v16 = tc.tile_pool(name="P_v16", bufs=1)
        p_v16 = P_v16.__enter__()
        v16P = p_v16.tile([128, LEN, 2], f16, tag="v16P", name="v16P")
        PH1 = tc.tile_pool(name="ph1", bufs=1)
        ph1 = PH1.__enter__()
        srcT = [ph1.tile([128, LEN], f16, tag=f"srcT{i}", name=f"srcT{i}")
                for i in range(2)]
        transpose_rows(srcT, src_full_d, LEN)
        for dlt in range(2):
            for j in range(_ceil(LEN, 512)):
                c0 = j * 512
                cn = min(512, LEN - c0)
                ps = mm_ps.tile([128, 512], f32, tag="mm")
                for kb in range(2):
                    nc.tensor.matmul(ps[:, :cn],
                                     w_val[kb][:, 128 * dlt:128 * dlt + 128],
                                     srcT[kb][:, c0:c0 + cn],
                                     start=(kb == 0), stop=(kb == 1))
                nc.scalar.activation(v16P[:, c0:c0 + cn, dlt], ps[:, :cn],
                                     AF.Identity, bias=bvalT[:, dlt:dlt + 1])
        PH1.__exit__(None, None, None)

        # ---------------- quad value table (all heads, built once) --------
        # entry e=(y0+1)*Wl+x slot k (f32 = channel pair):
        #   k0=(y0,x), k1=(y0,x+1), k2=(y0+1,x), k3=(y0+1,x+1); zero pads.
        VF = v16P[:].bitcast(f32)  # [128, LEN] packed channel pairs
        q16 = VPQ[:]
        for l in range(L):
            Hl, Wl = SPATIAL[l]
            hw, nel, lo = HWs[l], NEL[l], LOFF[l]
            lv = q16[:, QOFF[l]:QOFF[l + 1]].rearrange(
                "p (e s) -> p e s", s=4)
            nc.vector.memset(lv[:, 0:Wl, 0:2], 0.0)
            nc.vector.memset(lv[:, hw:nel, 2:4], 0.0)
            hi_n = hw if l < L - 1 else hw - 1
            if hi_n < hw:  # last entry of shortened hi-slots stays 0
                nc.vector.memset(lv[:, Wl + hi_n:nel, 1:2], 0.0)
                nc.vector.memset(lv[:, hi_n:hw, 3:4], 0.0)
            nc.scalar.copy(lv[:, Wl:nel, 0], VF[:, lo:lo + hw])
            nc.scalar.copy(lv[:, Wl:Wl + hi_n, 1],
                           VF[:, lo + 1:lo + 1 + hi_n])
            nc.scalar.copy(lv[:, 0:hw, 2], VF[:, lo:lo + hw])
            nc.scalar.copy(lv[:, 0:hi_n, 3],
                           VF[:, lo + 1:lo + 1 + hi_n])
        P_v16.__exit__(None, None, None)

        # ---------------- wrap idx tiles ----------------
        # head h's stream -> partitions 16h..16h+15 (one Q7 core per head);
        # split in two column groups so early gathers start after chunk 2.
        nc.gpsimd.load_library(library_config.ap_gather)
        WRAPP = tc.tile_pool(name="wrapp", bufs=1)
        wrapp = WRAPP.__enter__()
        wraps = {}
        CSPLIT = 96  # covers NCHUNK[0:3]; rest in second DMA
        for half in range(2):
            for l in range(L):
                for p in range(P):
                    if half == 0:
                        w = wrapp.tile([128, NQ // 16], i16,
                                       tag=f"wr{l}{p}", name=f"wr{l}{p}")
                        wraps[(l, p)] = w
                    w = wraps[(l, p)]
                    p0 = l * 32 + p * 8
                    src = e16[p0:p0 + 8].rearrange(
                        "h (w c) -> h w c", c=NQ // 16)
                    if half == 0:
                        nc.sync.dma_start(w[:, :CSPLIT], src[:, :, :CSPLIT])
                    else:
                        nc.sync.dma_start(w[:, CSPLIT:], src[:, :, CSPLIT:])

        # ---------------- gathers + combine ----------------
        GP = tc.tile_pool(name="gp", bufs=3)
        gp = GP.__enter__()
        WBP = tc.tile_pool(name="wbp", bufs=2)
        wbp = WBP.__enter__()
        MP = tc.tile_pool(name="mp", bufs=2)
        mp = MP.__enter__()
        ACC_PS = tc.tile_pool(name="acc_ps", bufs=1, space="PSUM")
        acc_ps = ACC_PS.__enter__()
        WB_PS = tc.tile_pool(name="wb_ps", bufs=1, space="PSUM")
        wb_ps = WB_PS.__enter__()
        accP16 = p_acc.tile([128, NQ, 2], f16, tag="accP16", name="accP16")

        for ci, cn in enumerate(NCHUNK):
            c0 = COFF[ci]
            ccols, cw0 = cn // 16, c0 // 16
            accs = [acc_ps.tile([128, 512], f32, tag=f"acc{d}",
                                name=f"acc{d}")
                    for d in range(2)]
            for l in range(L):
                for p in range(P):
                    gt = gp.tile([128, 2048], f32, tag="g")
                    nc.gpsimd.ap_gather(
                        gt[:, :4 * cn],
                        VPQ[:, QOFF[l]:QOFF[l + 1]],
                        wraps[(l, p)][:, cw0:cw0 + ccols],
                        channels=128, num_elems=NEL[l], d=4,
                        num_idxs=cn)
                    b64 = l // 2
                    sel = bselP[64 * b64:64 * b64 + 64, (l % 2) * 4 + p, :]
                    wb16 = wbp.tile([128, 4096], f16, tag="wb16")
                    for hs in range(0, cn, 256):
                        hn = min(256, cn - hs)
                        wb = wb_ps.tile([128, 1024], f32, tag="wb")
                        rsrc = W4[64 * b64:64 * b64 + 64,
                                  c0 + hs:c0 + hs + hn, :, :] \
                            .rearrange("h q r s -> h (q r s)")
                        for a in range(0, 4 * hn, 512):
                            an = min(512, 4 * hn - a)
                            nc.tensor.matmul(wb[:, a:a + an], sel,
                                             rsrc[:, a:a + an],
                                             start=True, stop=True)
                        # duplicate each weight to the channel-pair lanes
                        wv = wb16[:, 8 * hs:8 * (hs + hn)].rearrange(
                            "p (x s) -> p x s", s=2)
                        nc.scalar.copy(wv[:, :, 0], wb[:, :4 * hn])
                        nc.scalar.copy(wv[:, :, 1], wb[:, :4 * hn])
                    m = mp.tile([128, 4096], f16, tag="m")
                    nc.vector.tensor_tensor(
                        m[:, :8 * cn], gt[:, :4 * cn].bitcast(f16),
                        wb16[:, :8 * cn], op=AL.mult)
                    mv = m[:, :8 * cn].rearrange("p (q s) -> p q s", s=8)
                    # start zeroes the whole 2KB bank: only the very
                    # first matmul per acc bank starts the group
                    first = (l == 0 and p == 0)
                    last = (l == L - 1 and p == P - 1)
                    for d in range(2):
                        for k in range(4):
                            nc.tensor.matmul(
                                accs[d][:, :cn], ident16[:],
                                mv[:, :, 2 * k + d],
                                start=(first and k == 0),
                                stop=(last and k == 3))
            for d in range(2):
                nc.scalar.copy(accP16[:, c0:c0 + cn, d], accs[d][:, :cn])

        WB_PS.__exit__(None, None, None)
        ACC_PS.__exit__(None, None, None)
        MP.__exit__(None, None, None)
        WBP.__exit__(None, None, None)
        GP.__exit__(None, None, None)
        WRAPP.__exit__(None, None, None)
        P_w4.__exit__(None, None, None)
        P_vpq.__exit__(None, None, None)

        # ---------------- out-proj + residual + LN1 ----------------
        p_f = ctx.enter_context(tc.tile_pool(name="P_f", bufs=1))
        srcqT = [p_f.tile([128, NQ], f32, tag=f"srcqT{i}", name=f"srcqT{i}")
                 for i in range(2)]
        transpose_rows(srcqT, srcq_d, NQ)
        w1 = cstk(w1_d, 2, DFF, f16, pl=p_f)
        w2 = cstk(w2_d, 8, C, f16, pl=p_f)
        lnp = pool("lnp", 1)
        ln_ps = psum("ln_ps", 1)

        def layernorm_chunk(xT, gT, beT, dstT, c0j, cnj):
                psm = ln_ps.tile([1, 512], f32, tag="lnm", name="lnm")
                psv = ln_ps.tile([1, 512], f32, tag="lnv", name="lnv")
                sqc = [None, None]
                for i in range(2):
                    sqc[i] = lnp.tile([128, 512], f32, tag=f"sqc{i}",
                                      name=f"sqc{i}")
                    nc.vector.tensor_tensor(sqc[i][:, :cnj],
                                            xT[i][:, c0j:c0j + cnj],
                                            xT[i][:, c0j:c0j + cnj],
                                            op=AL.mult)
                for i in range(2):
                    nc.tensor.matmul(psm[:, :cnj], ones_col[:],
                                     xT[i][:, c0j:c0j + cnj],
                                     start=(i == 0), stop=(i == 1))
                for i in range(2):
                    nc.tensor.matmul(psv[:, :cnj], ones_col[:],
                                     sqc[i][:, :cnj],
                                     start=(i == 0), stop=(i == 1))
                mrow = lnp.tile([1, 512], f32, tag="mrow", name="mrow")
                vrow = lnp.tile([1, 512], f32, tag="vrow", name="vrow")
                nc.scalar.activation(mrow[:, :cnj], psm[:, :cnj], AF.Copy,
                                     scale=1.0 / C)
                nc.scalar.activation(vrow[:, :cnj], psv[:, :cnj], AF.Copy,
                                     scale=1.0 / C)
                msq = lnp.tile([1, 512], f32, tag="msq", name="msq")
                nc.vector.tensor_tensor(msq[:, :cnj], mrow[:, :cnj],
                                        mrow[:, :cnj], op=AL.mult)
                nc.vector.tensor_tensor(vrow[:, :cnj], vrow[:, :cnj],
                                        msq[:, :cnj], op=AL.subtract)
                nc.scalar.activation(vrow[:, :cnj], vrow[:, :cnj], AF.Sqrt,
                                     bias=c_eps1[:])
                rrow = lnp.tile([1, 512], f32, tag="rrow", name="rrow")
                nc.vector.reciprocal(rrow[:, :cnj], vrow[:, :cnj])
                psbm = ln_ps.tile([128, 512], f32, tag="lnbm", name="lnbm")
                psbr = ln_ps.tile([128, 512], f32, tag="lnbr", name="lnbr")
                nc.tensor.matmul(psbm[:, :cnj], ones1x128[:],
                                 mrow[:, :cnj], start=True, stop=True)
                nc.tensor.matmul(psbr[:, :cnj], ones1x128[:],
                                 rrow[:, :cnj], start=True, stop=True)
                for i in range(2):
                    t = lnp.tile([128, 512], f32, tag="lt", name="lt")
                    nc.vector.tensor_tensor(t[:, :cnj], xT[i][:, c0j:c0j + cnj],
                                            psbm[:, :cnj], op=AL.subtract)
                    nc.vector.tensor_tensor(t[:, :cnj], t[:, :cnj],
                                            psbr[:, :cnj], op=AL.mult)
                    nc.vector.scalar_tensor_tensor(
                        dstT[i][:, c0j:c0j + cnj], t[:, :cnj], gT[:, i:i + 1],
                        beT[:, i:i + 1].to_broadcast([128, cnj]),
                        op0=AL.mult, op1=AL.add)

        # per-chunk pipeline: out-proj -> LN1 -> FFN -> LN2 -> transpose,
        # no full-width barriers between stages
        xT = [p_f.tile([128, NQ], f32, tag=f"xT{i}", name=f"xT{i}") for i in range(2)]
        pre = [lnp.tile([128, NQ], f32, tag=f"pre{i}", name=f"pre{i}") for i in range(2)]
        xT16 = [p_f.tile([128, NQ], f16, tag=f"xT16_{i}", name=f"xT16_{i}")
                for i in range(2)]
        fpre = [lnp.tile([128, NQ], f32, tag=f"fpre{i}", name=f"fpre{i}") for i in range(2)]
        outT = [p_f.tile([128, NQ], f32, tag=f"outT{i}", name=f"outT{i}") for i in range(2)]
        hp = ctx.enter_context(tc.tile_pool(name="hp", bufs=2))
        for j in range(_ceil(NQ, 512)):
            c0j, cnj = j * 512, min(512, NQ - j * 512)
            for mb in range(2):
                ps = mm_ps.tile([128, 512], f32, tag="mm")
                for d in range(2):
                    nc.tensor.matmul(ps[:, :cnj],
                                     w_out16[d][:, 128 * mb:128 * mb + 128],
                                     accP16[:, c0j:c0j + cnj, d],
                                     start=(d == 0), stop=(d == 1))
                nc.scalar.activation(pre[mb][:, c0j:c0j + cnj], ps[:, :cnj],
                                     AF.Identity, bias=boutT[:, mb:mb + 1])
            for i in range(2):
                nc.vector.tensor_tensor(pre[i][:, c0j:c0j + cnj],
                                        pre[i][:, c0j:c0j + cnj],
                                        srcqT[i][:, c0j:c0j + cnj],
                                        op=AL.add)
            layernorm_chunk(pre, g1T, be1T, xT, c0j, cnj)
            for i in range(2):
                nc.vector.tensor_copy(xT16[i][:, c0j:c0j + cnj],
                                      xT[i][:, c0j:c0j + cnj])
            hts = []
            for mb in range(8):
                ps = mm_ps.tile([128, 512], f32, tag="mm")
                for kb in range(2):
                    nc.tensor.matmul(ps[:, :cnj],
                                     w1[kb][:, 128 * mb:128 * mb + 128],
                                     xT16[kb][:, c0j:c0j + cnj],
                                     start=(kb == 0), stop=(kb == 1))
                ht = hp.tile([128, 512], f16, tag=f"ht{mb}", name=f"ht{mb}")
                nc.scalar.activation(ht[:, :cnj], ps[:, :cnj],
                                     AF.Relu, bias=b1T[:, mb:mb + 1])
                hts.append(ht)
            for mb in range(2):
                ps = mm_ps.tile([128, 512], f32, tag="mm")
                for kb in range(8):
                    nc.tensor.matmul(ps[:, :cnj],
                                     w2[kb][:, 128 * mb:128 * mb + 128],
                                     hts[kb][:, :cnj],
                                     start=(kb == 0), stop=(kb == 7))
                nc.scalar.activation(fpre[mb][:, c0j:c0j + cnj], ps[:, :cnj],
                                     AF.Identity, bias=b2T[:, mb:mb + 1])
            for i in range(2):
                nc.vector.tensor_tensor(fpre[i][:, c0j:c0j + cnj],
                                        fpre[i][:, c0j:c0j + cnj],
                                        xT[i][:, c0j:c0j + cnj], op=AL.add)
            layernorm_chunk(fpre, g2T, be2T, outT, c0j, cnj)
            for r0 in range(c0j, c0j + cnj, 128):
                rn = min(128, c0j + cnj - r0)
                ot = rowp.tile([128, C], f32, tag="orow")
                for cb in range(2):
                    ps = tp_ps.tile([128, 128], f32, tag="tp")
                    nc.tensor.transpose(ps[:rn], outT[cb][:, r0:r0 + rn],
                                        ident[:])
                    nc.scalar.copy(ot[:rn, 128 * cb:128 * cb + 128], ps[:rn])
                nc.sync.dma_start(out_d[r0:r0 + rn], ot[:rn])

    nc.compile()
    return nc


def build_baseline_nc():
    """Same I/O signature, trivial work - for dispatch-overhead baseline."""
    nc = bacc.Bacc(None, target_bir_lowering=False, debug=False)
    ds = {}
    ds['src_full'] = nc.dram_tensor("src_full", [LEN, C], f32, kind="ExternalInput")
    ds['srcq'] = nc.dram_tensor("srcq", [NQ, C], f32, kind="ExternalInput")
    ds['posq'] = nc.dram_tensor("posq", [NQ, C], f32, kind="ExternalInput")
    ds['refq'] = nc.dram_tensor("refq", [NQ, 8], f32, kind="ExternalInput")
    ds['w_val'] = nc.dram_tensor("w_val", [C, C], f32, kind="ExternalInput")
    ds['bvalT'] = nc.dram_tensor("bvalT", [128, 2], f32, kind="ExternalInput")
    ds['w_offp'] = nc.dram_tensor("w_offp", [C, C], f32, kind="ExternalInput")
    ds['refsel'] = nc.dram_tensor("refsel", [16, C], f32, kind="ExternalInput")
    ds['w_attnp'] = nc.dram_tensor("w_attnp", [C, 128], f32, kind="ExternalInput")
    ds['b_attnp'] = nc.dram_tensor("b_attnp", [1, 128], f32, kind="ExternalInput")
    ds['w_out16'] = nc.dram_tensor("w_out16", [C, C], f32, kind="ExternalInput")
    ds['boutT'] = nc.dram_tensor("boutT", [128, 2], f32, kind="ExternalInput")
    ds['g1T'] = nc.dram_tensor("g1T", [128, 2], f32, kind="ExternalInput")
    ds['be1T'] = nc.dram_tensor("be1T", [128, 2], f32, kind="ExternalInput")
    ds['g2T'] = nc.dram_tensor("g2T", [128, 2], f32, kind="ExternalInput")
    ds['be2T'] = nc.dram_tensor("be2T", [128, 2], f32, kind="ExternalInput")
    ds['w1'] = nc.dram_tensor("w1", [C, DFF], f32, kind="ExternalInput")
    ds['b1T'] = nc.dram_tensor("b1T", [128, 8], f32, kind="ExternalInput")
    ds['w2'] = nc.dram_tensor("w2", [DFF, C], f32, kind="ExternalInput")
    ds['b2T'] = nc.dram_tensor("b2T", [128, 2], f32, kind="ExternalInput")
    ds['ident'] = nc.dram_tensor("ident", [128, 128], f32, kind="ExternalInput")
    ds['ident16'] = nc.dram_tensor("ident16", [128, 128], f32, kind="ExternalInput")
    ds['bsel16'] = nc.dram_tensor("bsel16", [128, 16, 128], f32, kind="ExternalInput")
    ds['pconst'] = nc.dram_tensor("pconst", [128, 5], f32, kind="ExternalInput")
    out_d = nc.dram_tensor("out", [NQ, C], f32, kind="ExternalOutput")
    with tile.TileContext(nc) as tc:
        with tc.tile_pool(name="p", bufs=2) as pl:
            for i in range(_ceil(NQ, 128)):
                r0 = i * 128
                rn = min(128, NQ - r0)
                t = pl.tile([128, C], f32, tag="t", name="t")
                nc.sync.dma_start(t[:rn], ds['srcq'][r0:r0 + rn])
                nc.sync.dma_start(out_d[r0:r0 + rn], t[:rn])
    nc.compile()
    return nc


# ======================= host side =======================

def _chp(p, d):
    """packed partition (p, half d) -> original channel"""
    return 32 * (p // 16) + 2 * (p % 16) + d


def _mk_bselP():
    b = np.zeros((128, 16, 128), np.float32)
    for l in range(4):
        for p in range(4):
            for h in range(8):
                r = 64 * (l // 2) + (l % 2) * 32 + p * 8 + h
                b[r, (l % 2) * 4 + p, 16 * h:16 * h + 16] = 1.0
    return b


def host_prep(inputs):
    """Build the 8 per-core input maps from full inputs."""
    src = np.asarray(inputs['src'], np.float32)
    pos = np.asarray(inputs['pos'], np.float32)
    ref = np.asarray(inputs['reference_points'], np.float32)
    vr = np.asarray(inputs['valid_ratios'], np.float32)

    refs = ref * vr[:, None, :, :]          # [B, Len, L, 2]

    # permuted column order m = comp*128 + l*32 + p*8 + h
    w_off = np.asarray(inputs['w_off'], np.float32)
    b_off = np.asarray(inputs['b_off'], np.float32)
    w_attn = np.asarray(inputs['w_attn'], np.float32)
    b_attn = np.asarray(inputs['b_attn'], np.float32)
    perm_off = np.zeros(256, np.int64)
    for comp in range(2):
        for l in range(L):
            for p in range(P):
                for h in range(H):
                    m = comp * 128 + l * 32 + p * 8 + h
                    perm_off[m] = ((h * L + l) * P + p) * 2 + comp
    w_offp = w_off[:, perm_off].copy()
    b_offp = b_off[perm_off].copy()
    perm_attn = np.zeros(128, np.int64)
    for l in range(L):
        for p in range(P):
            for h in range(H):
                perm_attn[l * 32 + p * 8 + h] = (h * L + l) * P + p
    w_attnp = w_attn[:, perm_attn].copy()
    b_attnp = b_attn[perm_attn].reshape(1, 128).copy()

    # refsel [16, 256]: rows j=(l*2+comp) -> grid scale; row 8 -> ones coeff
    refsel = np.zeros((16, 256), np.float32)
    for comp in range(2):
        for l in range(L):
            Hl, Wl = SPATIAL[l]
            norm = Wl if comp == 0 else Hl
            for p in range(P):
                for h in range(H):
                    m = comp * 128 + l * 32 + p * 8 + h
                    refsel[l * 2 + comp, m] = float(norm)
    refsel[8, :] = b_offp - 1.0 + SH

    pconst = np.zeros((128, 5), np.float32)
    for l in range(L):
        Hl, Wl = SPATIAL[l]
        for p in range(P):
            for h in range(H):
                r = l * 32 + p * 8 + h
                pconst[r] = [Wl, SH + Wl - 1, SH + Wl - 2, SH + Hl - 1,
                             (SH - 1) * Wl + SH]

    # channel-pair packed permutations
    w_val = np.asarray(inputs['w_val'], np.float32)
    b_val = np.asarray(inputs['b_val'], np.float32)
    w_out = np.asarray(inputs['w_out'], np.float32)
    w_valP = np.zeros_like(w_val)
    w_outP = np.zeros_like(w_out)
    bvalP = np.zeros((128, 2), np.float32)
    for d in range(2):
        for p in range(128):
            c = _chp(p, d)
            w_valP[:, d * 128 + p] = w_val[:, c]
            w_outP[d * 128 + p, :] = w_out[c, :]
            bvalP[p, d] = b_val[c]

    def t2(v):
        return np.ascontiguousarray(
            v.reshape(2, 128).T.astype(np.float32))

    common = {
        'w_val': w_valP,
        'bvalT': bvalP,
        'w_offp': w_offp, 'refsel': refsel,
        'w_attnp': w_attnp, 'b_attnp': b_attnp,
        'w_out16': w_outP,
        'boutT': t2(np.asarray(inputs['b_out'], np.float32)),
        'g1T': t2(np.asarray(inputs['g1'], np.float32)),
        'be1T': t2(np.asarray(inputs['be1'], np.float32)),
        'g2T': t2(np.asarray(inputs['g2'], np.float32)),
        'be2T': t2(np.asarray(inputs['be2'], np.float32)),
        'w1': np.asarray(inputs['w1'], np.float32),
        'b1T': np.ascontiguousarray(
            np.asarray(inputs['b1'], np.float32).reshape(8, 128).T),
        'w2': np.asarray(inputs['w2'], np.float32),
        'b2T': t2(np.asarray(inputs['b2'], np.float32)),
        'ident': np.eye(128, dtype=np.float32),
        'ident16': np.eye(128, dtype=np.float32),
        'bsel16': _mk_bselP(),
        'pconst': pconst,
    }
    in_maps = []
    for core in range(8):
        b, half = core // 2, core % 2
        q0 = half * NQ
        im = dict(common)
        im['src_full'] = np.ascontiguousarray(src[b])
        im['srcq'] = np.ascontiguousarray(src[b, q0:q0 + NQ])
        im['posq'] = np.ascontiguousarray(pos[b, q0:q0 + NQ])
        im['refq'] = np.ascontiguousarray(
            refs[b, q0:q0 + NQ].reshape(NQ, 8))
        in_maps.append(im)
    return in_maps


_CACHE = {}


def _get_runner():
    if 'run' in _CACHE:
        return _CACHE['run']
    import jax
    from jax.sharding import Mesh, PartitionSpec
    from jax.experimental.shard_map import shard_map
    from concourse.bass2jax import (_bass_exec_p, install_neuronx_cc_hook,
                                    partition_id_tensor)
    nc = build_nc()
    _CACHE['nc'] = nc
    install_neuronx_cc_hook()
    partition_name = (nc.partition_id_tensor.name
                      if nc.partition_id_tensor else None)
    in_names, out_names, out_avals = [], [], []
    for alloc in nc.m.functions[0].allocations:
        if not isinstance(alloc, mybir.MemoryLocationSet):
            continue
        name = alloc.memorylocations[0].name
        if alloc.kind == "ExternalInput":
            if name != partition_name:
                in_names.append(name)
        elif alloc.kind == "ExternalOutput":
            out_names.append(name)
            out_avals.append(jax.core.ShapedArray(
                tuple(alloc.tensor_shape), mybir.dt.np(alloc.dtype)))
    n_params = len(in_names)
    n_outs = len(out_avals)
    zero_outs = [np.zeros(a.shape, a.dtype) for a in out_avals]
    all_names = list(in_names) + out_names
    if partition_name is not None:
        all_names.append(partition_name)
    donate = tuple(range(n_params, n_params + n_outs))

    def _body(*args):
        operands = list(args)
        if partition_name is not None:
            operands.append(partition_id_tensor())
        outs = _bass_exec_p.bind(
            *operands, out_avals=tuple(out_avals), in_names=tuple(all_names),
            out_names=tuple(out_names), lowering_input_output_aliases=(),
            sim_require_finite=True, sim_require_nnan=True, nc=nc)
        return tuple(outs)

    devices = jax.devices()[:8]
    mesh = Mesh(np.asarray(devices), ("core",))
    jit = jax.jit(shard_map(_body, mesh=mesh,
                            in_specs=(PartitionSpec("core"),) * (n_params + n_outs),
                            out_specs=(PartitionSpec("core"),) * n_outs,
                            check_rep=False),
                  donate_argnums=donate, keep_unused=True)

    def run(in_maps):
        args = [np.concatenate([np.asarray(m[n]) for m in in_maps], axis=0)
                for n in in_names]
        args += [np.concatenate([z.copy() for _ in range(8)], axis=0)
                 for z in zero_outs]
        outs = jit(*args)
        res = [dict() for _ in range(8)]
        for n, o in zip(out_names, outs):
            o = np.asarray(o)
            per = o.shape[0] // 8
            for c in range(8):
                res[c][n] = o[c * per:(c + 1) * per]
        return res

    _CACHE['run'] = run
    return run


def kernel(**inputs):
    in_maps = host_prep(inputs)
    run = _get_runner()
    res = run(in_maps)
    out = np.zeros((B, LEN, C), np.float32)
    for core in range(8):
        b, half = core // 2, core % 2
        out[b, half * NQ:(half + 1) * NQ] = res[core]['out']
    # int32 preservation n/a: output is f32
    return out
